# revision 1
# baseline (speedup 1.0000x reference)
"""Trainium2 Bass kernel for nn_Decoder (2-layer x 2-cell LSTM decoder +
vocab projection), SPMD across 8 NeuronCores.

Architecture (per core; identical program, per-core data):
  - Transposed recurrence: gates computed as G^T chunks [128 gates, 64
    batch] with whh^T k-chunks as the stationary matmul operand and h^T as
    the 64-wide moving operand; h^T is produced directly in the layout the
    next step consumes (no DMA transposes anywhere). Gate order (i,f,o,g)
    so one sigmoid covers i,f,o per cell; per-step work is issued in
    engine stages (PE -> ACT -> DVE -> ACT -> DVE) across both cells to
    avoid head-of-line blocking on the depth-0 ACT/DVE queues.
  - gih0 (= x @ wih0^T + b) for steps 0..31 is generated locally
    (interleaved into the layer-0 loop); steps 32..63 are computed
    gate-sharded (512 gates/core) and AllGathered (bf16) while the gen
    region runs, so the collective never stalls the recurrence.
  - gih1 is computed gate-sharded from x1 blocks as layer 0 produces
    them and AllGathered in 3 chunks (24/24/16 steps) fired mid-loop.
  - Collective-output prefetches issue on the Pool/SWDGE queue: their
    waits must not sit in the shared HWDGE semaphore lanes, where they
    would deadlock unrelated DMAs (lane watermark WAR waits).
  - FC (vocab-sharded 4000 rows/core) is interleaved into the layer-1
    loop as half-blocks lagging the recurrence by 2 steps; logits are
    written bf16, and the host adds fc_b and reorders (s,b)->(b,s).
  - In PE-bound phases (layer-0 gen region, all of layer 1) the gih term
    is added onto the whh PSUM accumulation in-place by the DVE instead
    of being injected via identity matmuls, trading idle DVE time for
    ~55us of PE time. The chain-bound layer-0 AG region keeps the
    PE-side inject (a DVE stage there would lengthen the critical path).
"""
import json
import os
import sys

sys.path.insert(0, "/opt/trn_rl_repo")

import ml_dtypes
import numpy as np

import concourse.bass as bass
import concourse.tile as tile
from concourse import mybir
from concourse.bass_utils import run_bass_kernel_spmd

BF16 = ml_dtypes.bfloat16
V, E, H, B, S = 32000, 512, 512, 64, 64
R = S * B              # 4096 rows, s-major: r = 64*s + b
NC = 8
VS = V // NC           # 4000 vocab rows per core
GB = 512               # gate rows per core shard

F32 = mybir.dt.float32
BF = mybir.dt.bfloat16
F8 = mybir.dt.float8e4           # e4m3

AF = mybir.ActivationFunctionType

GEN_STEPS = 32
AG1_F8 = os.environ.get("BASS_AG1_F8", "") != ""
AG0_STEPS = S - GEN_STEPS        # steps covered by the gih0 AllGather
CH1 = [0, 24, 48, 64]            # gih1 AllGather chunk boundaries


# --------------------------------------------------------------------------
# walrus workaround: this build allows at most 2 sem waits per instruction.
def _split_excess_waits(bir_json):
    j = json.loads(bir_json)
    n = 0
    for fn in j.get("functions", []):
        for blk in fn.get("blocks", []):
            out = []
            for inst in blk.get("instructions", []):
                si = inst.get("sync_info")
                ow = (si or {}).get("on_wait") or []
                keep = 2 if inst.get("opcode") == "EventSemaphore" else 1
                if len(ow) > keep:
                    extra, rest = ow[:-keep], ow[-keep:]
                    for i in range(0, len(extra), 2):
                        n += 1
                        out.append({
                            "debug": inst.get("debug", 0),
                            "engine": inst["engine"],
                            "ins": [], "outs": [],
                            "name": f"WSPLIT-{n}",
                            "opcode": "EventSemaphore",
                            "sync_info": {"on_update": [],
                                          "on_wait": extra[i:i + 2]},
                        })
                    si["on_wait"] = rest
                out.append(inst)
            blk["instructions"] = out
    return json.dumps(j).encode()


def _install_shim():
    import concourse.bass2jax as b2j
    import concourse.bass_utils as bu
    if getattr(bu, "_wsplit_installed", False):
        return
    orig = bu.compile_bir_kernel

    def patched(bir_json, tmpdir, neff_name="file.neff"):
        return orig(_split_excess_waits(bir_json), tmpdir, neff_name)

    bu.compile_bir_kernel = patched
    bu._wsplit_installed = True
    b2j.compile_bir_kernel = patched


# --------------------------------------------------------------------------
def build_nc():
    nc = bass.Bass()

    xT_in = nc.dram_tensor("xT", [128, 5, R], BF, kind="ExternalInput")
    wih0s_in = nc.dram_tensor("wih0s", [128, 5, GB], BF, kind="ExternalInput")
    wih0f_in = nc.dram_tensor("wih0f", [128, 5, 4096], BF,
                              kind="ExternalInput")
    wih1s_in = nc.dram_tensor("wih1s", [128, 9, GB], BF, kind="ExternalInput")
    whh0_in = nc.dram_tensor("whh0T", [128, 4, 4096], BF, kind="ExternalInput")
    whh1_in = nc.dram_tensor("whh1T", [128, 4, 4096], BF, kind="ExternalInput")
    fcw_in = nc.dram_tensor("fcwT", [128, 8, VS], BF, kind="ExternalInput")
    hT0_in = nc.dram_tensor("hT0", [128, 16, 64], BF, kind="ExternalInput")
    cT0_in = nc.dram_tensor("cT0", [128, 16, 64], F32, kind="ExternalInput")
    eye_in = nc.dram_tensor("eye128", [128, 128], BF, kind="ExternalInput")
    ones_in = nc.dram_tensor("onesS", [128, 512], BF, kind="ExternalInput")
    out = nc.dram_tensor("out", [R, VS], BF, kind="ExternalOutput")

    if AG0_STEPS > 0:
        g0loc = nc.dram_tensor("g0loc", [4, AG0_STEPS, 128, 64], BF)
        g0all = nc.dram_tensor("g0all", [32, AG0_STEPS, 128, 64], BF,
                               addr_space="Shared")
    n_ch1 = len(CH1) - 1
    DT1 = F8 if AG1_F8 else BF
    g1loc = [nc.dram_tensor(f"g1loc{k}", [4, CH1[k + 1] - CH1[k], 128, 64],
                            DT1) for k in range(n_ch1)]
    g1all = [nc.dram_tensor(f"g1all{k}", [32, CH1[k + 1] - CH1[k], 128, 64],
                            DT1, addr_space="Shared") for k in range(n_ch1)]

    with tile.TileContext(nc) as tc:
        with tc.tile_pool(name="persist", bufs=1) as persist:
            eye = persist.tile([128, 128], BF)
            nc.sync.dma_start(eye[:], eye_in[:])
            eye8 = persist.tile([128, 128], F8)
            nc.vector.tensor_copy(eye8[:], eye[:])
            ones = persist.tile([128, 512], BF)
            nc.sync.dma_start(ones[:], ones_in[:])
            hTi = persist.tile([128, 16, 64], BF)
            nc.sync.dma_start(hTi[:], hT0_in[:])
            cT = persist.tile([128, 16, 64], F32)
            nc.sync.dma_start(cT[:], cT0_in[:])
            # ---- phase A: gihT0 shard GEMM (steps GEN_STEPS..S) + AG0 ----
            # xT and the full wih0 stay resident through the gen region of
            # phase B (pool closed there).
            abp = tc.tile_pool(name="phABw", bufs=1)
            abw = abp.__enter__()
            xTt = abw.tile([128, 5, R], BF)
            nc.sync.dma_start(xTt[:], xT_in[:])
            wih0f = abw.tile([128, 5, 4096], BF)
            nc.sync.dma_start(wih0f[:], wih0f_in[:])
            if AG0_STEPS > 0:
              with (
                tc.tile_pool(name="ph0w", bufs=1) as ph0w,
                tc.tile_pool(name="ph0s", bufs=3) as ph0s,
                tc.tile_pool(name="ph0p", bufs=3, space="PSUM") as ph0p,
              ):
                wih0s = ph0w.tile([128, 5, GB], BF)
                nc.sync.dma_start(wih0s[:], wih0s_in[:])
                r0 = 64 * GEN_STEPS
                for gc in range(4):
                    for rb in range(AG0_STEPS // 8):
                        ps = ph0p.tile([128, 512], F32, tag="ps")
                        for kc in range(5):
                            nc.tensor.matmul(
                                ps[:],
                                wih0s[:, kc, 128 * gc:128 * (gc + 1)],
                                xTt[:, kc,
                                    r0 + 512 * rb:r0 + 512 * (rb + 1)],
                                start=(kc == 0), stop=(kc == 4),
                            )
                        sb = ph0s.tile([128, 8, 64], BF, tag="sb")
                        nc.scalar.activation(
                            sb[:].rearrange("p s r -> p (s r)"), ps[:],
                            AF.Copy)
                        nc.sync.dma_start(
                            g0loc[gc, 8 * rb:8 * (rb + 1)].rearrange(
                                "s p r -> p s r"), sb[:])
              nc.gpsimd.collective_compute(
                  "AllGather", mybir.AluOpType.bypass,
                  ins=[g0loc[:]], outs=[g0all[:]],
                  replica_groups=[list(range(NC))],
              )

            # ---- recurrence loop (shared for both layers) ----------------
            def lstm_step(layer, s, whh_sb, injects, hsrcs, hdsts,
                          gadds=None):
                """Both cells of a layer, one step, issued in engine stages
                to avoid head-of-line blocking (ACT/DVE exec queues have
                depth 0). injects[c]: list of (region, lhsT, rhs) gih-inject
                matmuls; region slices a [128,16,64] psum tile (canonical
                i,f,o,g x hsub). gadds[c] (alternative to injects when the
                phase is PE-bound): gihT tile added on the DVE instead of
                injected via the PE. hsrcs[c](kc) -> [128,64] prev-h^T
                chunk. hdsts[c]: [128,4,64] slice for new h^T (bf16)."""
                pss, sigs, tgs, tc2s = {}, {}, {}, {}
                # stage 1: PE — whh accumulation (+ gih inject if not DVE)
                for c in range(2):
                    ps = psum_p.tile([128, 16, 64], F32, tag=f"ps{c}",
                                     bufs=1)
                    pss[c] = ps
                    if injects is not None:
                        for region, lhsT, rhs in injects[c](ps):
                            nc.tensor.matmul(
                                region, lhsT, rhs, start=True, stop=False,
                            )
                    for gc in range(16):
                        gcg = 16 * c + gc
                        for kc in range(4):
                            nc.tensor.matmul(
                                ps[:, gc, :],
                                whh_sb[:, kc, 128 * gcg:128 * (gcg + 1)],
                                hsrcs[c](kc),
                                start=(injects is None and kc == 0),
                                stop=(kc == 3),
                                skip_group_check=True,
                            )
                # stage 1.5: DVE — add gih onto the whh sum in-place in
                # PSUM; gadds[c] = list of (region-slicer, gih-tile-view)
                if gadds is not None:
                    for c in range(2):
                        for sl, gtv in gadds[c]:
                            nc.vector.tensor_add(sl(pss[c]), sl(pss[c]),
                                                 gtv)
                # stage 2: ACT — gate nonlinearities (i,f,o sigmoid; g tanh)
                for c in range(2):
                    sig = ep.tile([128, 12, 64], F32, tag=f"sig{c}")
                    nc.scalar.activation(sig[:], pss[c][:, 0:12, :],
                                         AF.Sigmoid)
                    sigs[c] = sig
                    tg = ep.tile([128, 4, 64], F32, tag=f"tg{c}")
                    nc.scalar.activation(tg[:], pss[c][:, 12:16, :], AF.Tanh)
                    tgs[c] = tg
                # stage 3: DVE — cell-state update
                for c in range(2):
                    cell = 2 * layer + c
                    csl = cT[:, 4 * cell:4 * (cell + 1), :]
                    t1 = ep.tile([128, 4, 64], F32, tag=f"t1{c}")
                    nc.vector.tensor_mul(t1[:], sigs[c][:, 4:8, :], csl)
                    t2 = ep.tile([128, 4, 64], F32, tag=f"t2{c}")
                    nc.vector.tensor_mul(t2[:], sigs[c][:, 0:4, :], tgs[c][:])
                    nc.vector.tensor_add(csl, t1[:], t2[:])
                # stage 4: ACT — tanh(c)
                for c in range(2):
                    cell = 2 * layer + c
                    tc2 = ep.tile([128, 4, 64], F32, tag=f"tc2{c}")
                    nc.scalar.activation(
                        tc2[:], cT[:, 4 * cell:4 * (cell + 1), :], AF.Tanh)
                    tc2s[c] = tc2
                # stage 5: DVE — h = sig(o) * tanh(c)
                for c in range(2):
                    nc.vector.tensor_mul(hdsts[c], sigs[c][:, 8:12, :],
                                         tc2s[c][:])

            # ---- phase B: layer-0 recurrence + gihT1 shard GEMM ----------
            sh1_units = [(gc, rb) for rb in range(8) for gc in range(4)]

            with (
                tc.tile_pool(name="phBw", bufs=1) as bwp,
                tc.tile_pool(name="phBg", bufs=4) as gp,
                tc.tile_pool(name="phBe", bufs=2) as ep,
                tc.tile_pool(name="phBx", bufs=2) as xp,
                tc.tile_pool(name="phBs", bufs=3) as shp,
                tc.tile_pool(name="phBp", bufs=1, space="PSUM") as psum_p,
                tc.tile_pool(name="phBq", bufs=3, space="PSUM") as psum_q,
            ):
                whh0 = bwp.tile([128, 4, 4096], BF)
                nc.sync.dma_start(whh0[:], whh0_in[:])
                wih1s = bwp.tile([128, 9, GB], BF)
                nc.sync.dma_start(wih1s[:], wih1s_in[:])
                x1blks = {}
                gts = {}     # (s, c) -> f8 gihT tile (AG region)
                gens = {}    # (s, c, half) -> bf16 gihT tile (gen region)

                def prefetch0(s):
                    if not (GEN_STEPS <= s < S):
                        return
                    for c in range(2):
                        gt = gp.tile([128, 16, 64], BF, tag=f"g{c}")
                        nc.gpsimd.dma_start(
                            gt[:],
                            g0all[16 * c:16 * (c + 1), s - GEN_STEPS]
                            .rearrange("gc p r -> p gc r"))
                        gts[(s, c)] = gt

                def gen0_unit(s, c, h):
                    """Locally compute gihT0 for (step s, cell c, gate-half
                    h) from xT — covers steps the AG0 doesn't."""
                    psg = psum_q.tile([128, 8, 64], F32, tag="sh")
                    for gc in range(8):
                        gcg = 16 * c + 8 * h + gc
                        for kc in range(5):
                            nc.tensor.matmul(
                                psg[:, gc, :],
                                wih0f[:, kc, 128 * gcg:128 * (gcg + 1)],
                                xTt[:, kc, 64 * s:64 * (s + 1)],
                                start=(kc == 0), stop=(kc == 4),
                            )
                    gt = gp.tile([128, 8, 64], BF, tag=f"gen{c}{h}")
                    nc.scalar.activation(
                        gt[:].rearrange("p g r -> p (g r)"),
                        psg[:].rearrange("p g r -> p (g r)"), AF.Copy)
                    gens[(s, c, h)] = gt

                def shard1_unit(gc, rb):
                    ps1 = psum_q.tile([128, 8, 64], F32, tag="sh")
                    p1v = ps1[:].rearrange("p s r -> p (s r)")
                    blk = x1blks[rb]
                    for kc in range(8):
                        nc.tensor.matmul(
                            p1v,
                            wih1s[:, kc, 128 * gc:128 * (gc + 1)],
                            blk[:, kc, :].rearrange("p s r -> p (s r)"),
                            start=(kc == 0), stop=False,
                        )
                    nc.tensor.matmul(
                        p1v,
                        wih1s[:, 8, 128 * gc:128 * (gc + 1)],
                        ones[:],
                        start=False, stop=True,
                    )
                    sb1 = shp.tile([128, 8, 64], DT1, tag="sb1")
                    nc.scalar.activation(
                        sb1[:].rearrange("p s r -> p (s r)"), p1v, AF.Copy)
                    ck = next(k for k in range(n_ch1)
                              if CH1[k] <= 8 * rb < CH1[k + 1])
                    cs = 8 * rb - CH1[ck]
                    nc.scalar.dma_start(
                        g1loc[ck][gc, cs:cs + 8].rearrange(
                            "s p r -> p s r"), sb1[:])

                def mk_injects(s):
                    if s < GEN_STEPS:
                        return None
                    injs = []
                    for c in range(2):
                        gt = gts.pop((s, c))
                        injs.append(lambda ps, gt=gt: [
                            (ps[:, 8 * h:8 * (h + 1), :].rearrange(
                                "p g r -> p (g r)"),
                             eye[:],
                             gt[:, 8 * h:8 * (h + 1), :].rearrange(
                                "p g r -> p (g r)"))
                            for h in range(2)
                        ])
                    return injs

                def mk_gadds0(s):
                    if s >= GEN_STEPS:
                        return None
                    out = []
                    for c in range(2):
                        ga = gens.pop((s, c, 0))
                        gb = gens.pop((s, c, 1))
                        out.append([
                            (lambda t: t[:, 0:8, :], ga[:]),
                            (lambda t: t[:, 8:16, :], gb[:]),
                        ])
                    return out

                for s in range(2):
                    prefetch0(s)
                    for c in range(2):
                        for h in range(2):
                            gen0_unit(s, c, h)
                bgq = list(sh1_units)
                ag1_fired = 0
                for s in range(S):
                    prefetch0(s + 2)
                    if s + 2 < GEN_STEPS:
                        for c in range(2):
                            for h in range(2):
                                gen0_unit(s + 2, c, h)
                    rb, ri = s // 8, s % 8
                    if ri == 0:
                        x1b = xp.tile([128, 8, 8, 64], BF, tag="x1")
                        x1blks[rb] = x1b
                    x1b = x1blks[rb]
                    if s == 0:
                        hsrcs = [lambda kc, c=c: hTi[:, 4 * c + kc, :]
                                 for c in range(2)]
                    else:
                        pb = x1blks[(s - 1) // 8]
                        pi = (s - 1) % 8
                        hsrcs = [lambda kc, c=c: pb[:, 4 * c + kc, pi, :]
                                 for c in range(2)]
                    lstm_step(
                        0, s, whh0, mk_injects(s), hsrcs,
                        [x1b[:, 4 * c:4 * (c + 1), ri, :] for c in range(2)],
                        gadds=mk_gadds0(s))
                    # background: gihT1 shard units (4 per 8 steps), once
                    # their x1 block is complete; fire AG1 chunks as soon
                    # as both their blocks are written out
                    if (s % 2 == 1 or s >= GEN_STEPS) and bgq \
                            and bgq[0][1] < rb:
                        shard1_unit(*bgq.pop(0))
                        done_rb = (bgq[0][1] if bgq else 8)
                        while (ag1_fired < n_ch1 and
                               CH1[ag1_fired + 1] <= 8 * done_rb):
                            k = ag1_fired
                            nc.gpsimd.collective_compute(
                                "AllGather", mybir.AluOpType.bypass,
                                ins=[g1loc[k][:]], outs=[g1all[k][:]],
                                replica_groups=[list(range(NC))],
                            )
                            ag1_fired += 1
                for u in bgq:
                    shard1_unit(*u)
            abp.__exit__(None, None, None)
            for k in range(ag1_fired, n_ch1):
                nc.gpsimd.collective_compute(
                    "AllGather", mybir.AluOpType.bypass,
                    ins=[g1loc[k][:]], outs=[g1all[k][:]],
                    replica_groups=[list(range(NC))],
                )

            # ---- phase C: layer-1 recurrence + FC ------------------------
            with (
                tc.tile_pool(name="phCw", bufs=1) as fwp,
                tc.tile_pool(name="phCg", bufs=4) as gp,
                tc.tile_pool(name="phCe", bufs=2) as ep,
                tc.tile_pool(name="phCh", bufs=3) as hp,
                tc.tile_pool(name="phCo", bufs=3) as fop,
                tc.tile_pool(name="phCp", bufs=1, space="PSUM") as psum_p,
                tc.tile_pool(name="phCf", bufs=3, space="PSUM") as psum_f,
            ):
                whh1 = fwp.tile([128, 4, 4096], BF)
                nc.sync.dma_start(whh1[:], whh1_in[:])
                fcw = fwp.tile([128, 8, VS], BF)
                for kc in range(8):
                    nc.sync.dma_start(fcw[:, kc, :], fcw_in[:, kc, :])

                hblks = {}
                gts = {}

                def prefetch1(s):
                    if s >= S:
                        return
                    ck = next(k for k in range(n_ch1)
                              if CH1[k] <= s < CH1[k + 1])
                    for c in range(2):
                        gt = gp.tile([128, 16, 64], DT1, tag=f"g{c}")
                        nc.gpsimd.dma_start(
                            gt[:],
                            g1all[ck][16 * c:16 * (c + 1),
                                      s - CH1[ck]].rearrange(
                                "gc p r -> p gc r"))
                        gts[(s, c)] = gt

                def mk_gadds1(s):
                    return [[(lambda t: t[:], gts.pop((s, c))[:])]
                            for c in range(2)]

                obs = {}

                def fc_half(m, hf):
                    hb = hblks[m]
                    if hf == 0:
                        obs[m] = fop.tile([128, VS], BF, tag="ob", name="ob")
                    ob = obs[m]
                    for n in range(4 * hf, 4 * hf + 4):
                        psf = psum_f.tile([128, 500], F32, tag="fc")
                        for kc in range(8):
                            nc.tensor.matmul(
                                psf[:],
                                hb[:, kc, :].rearrange("p s r -> p (s r)"),
                                fcw[:, kc, 500 * n:500 * (n + 1)],
                                start=(kc == 0), stop=(kc == 7),
                            )
                        nc.vector.tensor_copy(ob[:, 500 * n:500 * (n + 1)],
                                              psf[:])
                    if hf == 1:
                        nc.sync.dma_start(out[128 * m:128 * (m + 1), :],
                                          ob[:])
                        obs.pop(m)

                prefetch1(0)
                prefetch1(1)
                for s in range(S):
                    prefetch1(s + 2)
                    m, mi = s // 2, s % 2
                    if mi == 0:
                        hb = hp.tile([128, 8, 2, 64], BF, tag="hb")
                        hblks[m] = hb
                    hb = hblks[m]
                    if s == 0:
                        hsrcs = [lambda kc, c=c: hTi[:, 8 + 4 * c + kc, :]
                                 for c in range(2)]
                    else:
                        pb = hblks[(s - 1) // 2]
                        pi = (s - 1) % 2
                        hsrcs = [lambda kc, c=c: pb[:, 4 * c + kc, pi, :]
                                 for c in range(2)]
                    lstm_step(
                        1, s, whh1, None, hsrcs,
                        [hb[:, 4 * c:4 * (c + 1), mi, :] for c in range(2)],
                        gadds=mk_gadds1(s))
                    # FC halves lag the recurrence by 2 steps so they never
                    # wait on a just-produced h (block m: steps 2m+2, 2m+3)
                    if s >= 2:
                        bm, bh = (s - 2) // 2, s % 2
                        fc_half(bm, bh)
                        if bm >= 2:
                            hblks.pop(bm - 2, None)
                fc_half(31, 0)
                fc_half(31, 1)
    return nc


_NC_CACHE = None

# canonical gate reorder: torch (i,f,g,o) -> kernel (i,f,o,g)
_GPERM = np.concatenate([
    np.arange(0, 1024),            # i, f
    np.arange(1536, 2048),         # o
    np.arange(1024, 1536),         # g
])


def _pack_inputs(hidden_state, cell_state, Y, emb, w_ih_l0, w_hh_l0, b_ih_l0,
                 b_hh_l0, w_ih_l1, w_hh_l1, b_ih_l1, b_hh_l1, fc_w, fc_b):
    idx_seq = np.concatenate([Y[:, 1:2], Y[:, :-1]], axis=1)  # (B,S)
    idx_flat = idx_seq.T.reshape(-1).astype(np.int64)          # r = 64s + b
    x_all = np.asarray(emb, np.float32)[idx_flat]              # (R, E)

    def packT(w, kchunks, extra_row=None):
        gdim, kk = w.shape
        kc_data = kk // 128
        outp = np.zeros((128, kchunks, gdim), BF16)
        for kc in range(kc_data):
            outp[:, kc, :] = w[:, 128 * kc:128 * (kc + 1)].T.astype(BF16)
        if extra_row is not None:
            outp[0, kc_data, :] = extra_row.astype(BF16)
        return outp

    def gcat(w2):
        # stack both cells' weights, gate order (i,f,o,g) within each cell
        return np.vstack([np.asarray(w2[0], np.float32)[_GPERM],
                          np.asarray(w2[1], np.float32)[_GPERM]])

    def bcat(b2):
        bb = np.asarray(b2, np.float32)
        return np.concatenate([bb[0][_GPERM], bb[1][_GPERM]])

    b0_cat = bcat(b_ih_l0 + b_hh_l0)
    b1_cat = bcat(b_ih_l1 + b_hh_l1)
    wih0_cat = gcat(w_ih_l0)
    wih1_cat = gcat(w_ih_l1)
    whh0_cat = gcat(w_hh_l0)
    whh1_cat = gcat(w_hh_l1)

    xT = np.zeros((128, 5, R), BF16)
    for kc in range(4):
        xT[:, kc, :] = x_all[:, 128 * kc:128 * (kc + 1)].T.astype(BF16)
    xT[0, 4, :] = BF16(1.0)

    wih0f = packT(wih0_cat, 5, b0_cat)
    whh0T = packT(whh0_cat, 4)
    whh1T = packT(whh1_cat, 4)

    hT0 = np.zeros((128, 16, 64), BF16)
    cT0 = np.zeros((128, 16, 64), np.float32)
    hs = np.asarray(hidden_state, np.float32)
    cs = np.asarray(cell_state, np.float32)
    for cell in range(4):
        for k in range(4):
            hT0[:, 4 * cell + k, :] = hs[cell][:, 128 * k:128 * (k + 1)].T
            cT0[:, 4 * cell + k, :] = cs[cell][:, 128 * k:128 * (k + 1)].T
    eye128 = np.eye(128, dtype=np.float32).astype(BF16)
    onesS = np.zeros((128, 512), BF16)
    onesS[0, :] = BF16(1.0)

    fc_w = np.asarray(fc_w, np.float32)
    ins = []
    for j in range(NC):
        gsl = slice(GB * j, GB * (j + 1))
        wih0s_j = packT(wih0_cat[gsl], 5, b0_cat[gsl])
        wih1s_j = packT(wih1_cat[gsl], 9, b1_cat[gsl])
        fcs = fc_w[VS * j:VS * (j + 1)]           # (4000, 1024)
        fcwT_j = np.zeros((128, 8, VS), BF16)
        for k in range(8):
            fcwT_j[:, k, :] = fcs[:, 128 * k:128 * (k + 1)].T.astype(BF16)
        ins.append({
            "xT": xT, "wih0s": wih0s_j, "wih0f": wih0f,
            "wih1s": wih1s_j,
            "whh0T": whh0T, "whh1T": whh1T, "fcwT": fcwT_j,
            "hT0": hT0, "cT0": cT0, "eye128": eye128, "onesS": onesS,
        })
    return ins


def kernel(**inputs):
    global _NC_CACHE
    _install_shim()
    if _NC_CACHE is None:
        _NC_CACHE = build_nc()
    nc = _NC_CACHE
    in_maps = _pack_inputs(**inputs)
    res = run_bass_kernel_spmd(nc, in_maps, list(range(NC)))
    parts = [np.asarray(res.results[j]["out"], np.float32)
             for j in range(NC)]
    logits = np.concatenate(parts, axis=1)          # (R, V), r = 64s+b
    logits = logits.reshape(S, B, V).transpose(1, 0, 2).reshape(B * S, V)
    logits = logits + np.asarray(inputs["fc_b"], np.float32)[None, :]
    return logits.astype(np.float32)

